# revision 6
# baseline (speedup 1.0000x reference)
"""Trainium2 Bass kernel for nn_CALayer (FFT-magnitude channel attention).

Math per (b, c) image X [256, 256] (real):
  F(p, q) = 2D DFT;  y[b,c] = mean over the centered (fftshifted) 100x100
  low-frequency crop of |F|;  s = sigmoid(w2 @ relu(w1 @ y + b1) + b2);
  out = x * s[:, :, None, None].

Implementation: DFT-as-matmul with Hermitian reduction. Since X is real,
|F(-p,-q)| = |F(p,q)|, so only p in 0..50 (51 rows) and q in -50..50
(101 cols) of the spectrum are computed, and the crop sum over
p,q in [-50, 49]^2 is recovered as two separable window sums:
  S = sum_{q in -50..49} sum_{p in 0..49} |F| + sum_{q in -49..50} sum_{p in 1..50} |F|.

Dataflow per core (2 batches x 64 channels), built for continuous DMA:
  - x arrives via SWDGE cast-DMA (fp32 HBM -> fp16 SBUF), h-major layout
    h = 2p + k so HBM reads are 2 KiB contiguous per (p, c). All 16 group
    tiles (2 batches x 8 groups, fp16) are resident at once, so the input
    stream never stalls on SBUF space and output DMAs overlap it.
  - step A (PE): U^T[w, p] = X^T @ Wu (image as stationary operand, 104 cols)
  - step B (PE): F^T[q, (ch, p)] = Wv^T @ U^T  (4 real matmuls per part)
  - mag: Fr^2 (ACT) , Fi^2 + add (DVE), sqrt (ACT)
  - crop sum: indicator matmuls over q with partition-replicated indicator
    columns, so the window sums land on ALL 128 partitions and the whole
    SE block runs partition-redundant on DVE/ACT (no transposes, no
    broadcast matmul, no cross-engine ping-pong).
  - scale: one DVE tensor_mul per group with s broadcast along the free dim
    into fp32 staging tiles, DMA out (HWDGE).

Sharding: pure data parallel over batch: core i handles batches 2i, 2i+1.
"""

import os
import sys

for _p in (
    "/root/.axon_site",
    "/root/.axon_site/_ro/trn_rl_repo",
    "/root/.axon_site/_ro/pypackages",
    "/opt/trn_rl_repo",
):
    if os.path.isdir(_p) and _p not in sys.path:
        sys.path.append(_p)

import numpy as np

import concourse.bacc as bacc
import concourse.mybir as mybir
import concourse.tile as tile
from concourse.bass_utils import run_bass_kernel_spmd

N_CORES = 8
B, C, H, W = 16, 64, 256, 256
BPC = B // N_CORES  # batches per core
CROP = 50
NP_ = 51   # p = 0..50
NQ = 101   # q = -50..50
NA = 104   # step-A out cols: [cos 51 | pad | -sin 51 | pad]
GS = 8     # channels per group
NG = C // GS
MID = 4    # SE bottleneck width
F32 = mybir.dt.float32
F16 = mybir.dt.float16
AF = mybir.ActivationFunctionType
ALU = mybir.AluOpType


def _build_consts(w1, b1, w2, b2):
    h_idx = np.arange(H)
    ang_p = 2 * np.pi * np.outer(h_idx, np.arange(NP_)) / H
    wu = np.zeros((H, NA), np.float32)
    wu[:, 0:NP_] = np.cos(ang_p)            # Ur block cols 0..50, col 51 zero pad
    wu[:, 52:52 + NP_] = -np.sin(ang_p)     # Ui block cols 52..102, col 103 zero pad
    ang_q = 2 * np.pi * np.outer(h_idx, np.arange(-CROP, CROP + 1)) / W
    cq = np.cos(ang_q).astype(np.float32)
    sq = np.sin(ang_q).astype(np.float32)
    wv = np.ascontiguousarray(np.concatenate([cq, sq, -sq], axis=1))  # [256, 303]
    # q-window indicator columns, replicated so the crop sums land on all
    # 128 output partitions (drives the partition-redundant SE block).
    r1 = np.zeros((NQ, 2 * 128), np.float32)
    r1[0:100, 0:128] = 1.0  # q in -50..49
    r1[1:101, 128:256] = 1.0  # q in -49..50
    w1s = (np.asarray(w1, np.float32) / 1e4)          # [4, C], fold mean /1e4
    w2s = np.asarray(w2, np.float32)                  # [C, 4]
    return {
        "wu": wu.astype(np.float16),
        "wv": wv.astype(np.float16),
        "r1ind": r1.astype(np.float16),
        "w1r": np.ascontiguousarray(np.tile(w1s[None], (128, 1, 1))),        # [128,4,C]
        "b1r": np.ascontiguousarray(np.tile(b1.astype(np.float32)[None], (128, 1))),
        "w2r": np.ascontiguousarray(np.tile(w2s.T[None], (128, 1, 1))),      # [128,4,C]
        "b2r": np.ascontiguousarray(np.tile(b2.astype(np.float32)[None], (128, 1))),
    }


def _build_nc():
    nc = bacc.Bacc("TRN2", target_bir_lowering=False, debug=False)
    x_d = nc.dram_tensor("x", [BPC, C, H, W], F32, kind="ExternalInput").ap()
    out_d = nc.dram_tensor("out", [BPC, C, H, W], F32, kind="ExternalOutput").ap()
    wu_d = nc.dram_tensor("wu", [H, NA], F16, kind="ExternalInput").ap()
    wv_d = nc.dram_tensor("wv", [W, 303], F16, kind="ExternalInput").ap()
    r1_d = nc.dram_tensor("r1ind", [NQ, 256], F16, kind="ExternalInput").ap()
    w1r_d = nc.dram_tensor("w1r", [128, MID, C], F32, kind="ExternalInput").ap()
    b1r_d = nc.dram_tensor("b1r", [128, MID], F32, kind="ExternalInput").ap()
    w2r_d = nc.dram_tensor("w2r", [128, MID, C], F32, kind="ExternalInput").ap()
    b2r_d = nc.dram_tensor("b2r", [128, C], F32, kind="ExternalInput").ap()

    with tile.TileContext(nc) as tc:
        with (
            tc.tile_pool(name="consts", bufs=1) as cpool,
            tc.tile_pool(name="xf", bufs=BPC * NG) as xpool,
            tc.tile_pool(name="stg", bufs=3) as spool,
            tc.tile_pool(name="work", bufs=2) as wpool,
            tc.tile_pool(name="psA", bufs=2, space="PSUM") as pA,
            tc.tile_pool(name="psB", bufs=1, space="PSUM") as pB,
            tc.tile_pool(name="psS", bufs=1, space="PSUM") as pS,
        ):
            # wu rows follow the x load layout h = 2p + k.
            wu_sb = cpool.tile([128, 2, NA], F16, name="wu_sb")
            nc.sync.dma_start(wu_sb[:], wu_d.rearrange("(p k) n -> p k n", k=2))
            # wv rows follow step A's output layout w = 128k + p.
            wv_sb = cpool.tile([128, 2, 303], F16, name="wv_sb")
            nc.sync.dma_start(wv_sb[:], wv_d.rearrange("(k p) n -> p k n", p=128))
            r1_sb = cpool.tile([NQ, 256], F16, name="r1_sb")
            nc.sync.dma_start(r1_sb[:], r1_d[:])
            w1r_sb = cpool.tile([128, MID, C], F32, name="w1r_sb")
            nc.sync.dma_start(w1r_sb[:], w1r_d[:])
            b1r_sb = cpool.tile([128, MID], F32, name="b1r_sb")
            nc.sync.dma_start(b1r_sb[:], b1r_d[:])
            w2r_sb = cpool.tile([128, MID, C], F32, name="w2r_sb")
            nc.sync.dma_start(w2r_sb[:], w2r_d[:])
            b2r_sb = cpool.tile([128, C], F32, name="b2r_sb")
            nc.sync.dma_start(b2r_sb[:], b2r_d[:])

            # All 16 group tiles are resident (fp16): every load is issued
            # up-front so the input stream runs back-to-back from t=0.
            # SWDGE casts fp32 -> fp16 in the DMA datapath.
            def load_group(b, g):
                t = xpool.tile([128, GS, 2, W], F16, name=f"xf_{b}_{g}", tag="xf")
                src = x_d[b].rearrange("c (p k) w -> p c k w", p=128)[
                    :, GS * g:GS * (g + 1), :, :
                ]
                nc.gpsimd.dma_start(t[:], src)
                return t

            xt = {}
            for b in range(BPC):
                for g in range(NG):
                    xt[b, g] = load_group(b, g)

            for b in range(BPC):
                y_sb = wpool.tile([128, C], F32, name="y_sb", tag="y")

                for g in range(NG):
                    xg = xt[b, g]
                    # ---- step A: U^T = X^T @ Wu, 4 channels per PSUM tile
                    u_sb = wpool.tile([128, GS * 2 * NA], F16, name="u_sb", tag="u")
                    for cblk in range(2):
                        psA = pA.tile([128, 4, 2, NA], F32, name="psA", tag="uA")
                        for jj in range(4):
                            j = 4 * cblk + jj
                            for wk in range(2):
                                for kk in range(2):
                                    nc.tensor.matmul(
                                        psA[:, jj, wk, :],
                                        xg[:, j, kk, 128 * wk:128 * (wk + 1)],
                                        wu_sb[:, kk, :],
                                        start=(kk == 0),
                                        stop=(kk == 1),
                                    )
                        nc.vector.tensor_copy(
                            u_sb[:, cblk * 4 * 2 * NA:(cblk + 1) * 4 * 2 * NA],
                            psA[:],
                        )

                    # ---- step B: F^T[q, (ch, p)] with complex arithmetic
                    psB = pB.tile([NQ, 1024], F32, name="psB", tag="fB")
                    fr = psB[:, 0:416]
                    fi = psB[:, 512:928]
                    u3 = u_sb.rearrange("p (c x) -> p c x", c=GS)
                    fr_terms, fi_terms = [], []
                    for k in range(2):
                        ur = u3[:, :, 104 * k:104 * k + 52]
                        ui = u3[:, :, 104 * k + 52:104 * k + 104]
                        ck = wv_sb[:, k, 0:101]
                        sk = wv_sb[:, k, 101:202]
                        snk = wv_sb[:, k, 202:303]
                        fr_terms += [(ck, ur), (sk, ui)]
                        fi_terms += [(ck, ui), (snk, ur)]
                    for i, (lhsT, rhs) in enumerate(fr_terms):
                        nc.tensor.matmul(fr, lhsT, rhs, start=(i == 0), stop=(i == 3))
                    for i, (lhsT, rhs) in enumerate(fi_terms):
                        nc.tensor.matmul(fi, lhsT, rhs, start=(i == 0), stop=(i == 3))

                    # ---- |F| = sqrt(Fr^2 + Fi^2): squares on ACT (DVE cannot
                    # read the same PSUM operand twice), add on DVE
                    m2 = wpool.tile([NQ, 416], F32, name="m2", tag="m2")
                    m2b = wpool.tile([NQ, 416], F32, name="m2b", tag="m2b")
                    nc.scalar.square(m2[:], fr)
                    nc.scalar.square(m2b[:], fi)
                    nc.vector.tensor_add(m2[:], m2[:], m2b[:])
                    mag = wpool.tile([NQ, 416], F16, name="mag", tag="mag")
                    nc.scalar.sqrt(mag[:], m2[:])

                    # ---- crop sum on all 128 partitions (replicated indicator)
                    mag3 = mag.rearrange("p (c x) -> p c x", c=GS)
                    g2_ps = pS.tile([128, 1024], F32, name="g2_ps", tag="G")
                    nc.tensor.matmul(
                        g2_ps[:, 0:400], r1_sb[:, 0:128], mag3[:, :, 0:50],
                        start=True, stop=True,
                    )
                    nc.tensor.matmul(
                        g2_ps[:, 512:912], r1_sb[:, 128:256], mag3[:, :, 1:51],
                        start=True, stop=True,
                    )
                    gred = wpool.tile([128, GS], F32, name="gred", tag="gred")
                    ga = g2_ps[:, 0:400].rearrange("p (c x) -> p c x", c=GS)
                    gb = g2_ps[:, 512:912].rearrange("p (c x) -> p c x", c=GS)
                    nc.vector.reduce_sum(gred[:], ga, axis=mybir.AxisListType.X)
                    gredb = wpool.tile([128, GS], F32, name="gredb", tag="gredb")
                    nc.vector.reduce_sum(gredb[:], gb, axis=mybir.AxisListType.X)
                    nc.vector.tensor_add(
                        y_sb[:, GS * g:GS * (g + 1)], gred[:], gredb[:]
                    )

                # ---- SE block, partition-redundant on DVE/ACT
                # h = relu(w1 @ (y/1e4) + b1); s = sigmoid(w2 @ h + b2)
                ttmp = wpool.tile([128, MID, C], F32, name="ttmp", tag="se_t")
                h4 = wpool.tile([128, MID], F32, name="h4", tag="se_h")
                ybc = y_sb.unsqueeze(1).broadcast_to((128, MID, C))
                nc.vector.tensor_mul(ttmp[:], w1r_sb[:], ybc)
                nc.vector.reduce_sum(h4[:], ttmp[:], axis=mybir.AxisListType.X)
                nc.vector.tensor_add(h4[:], h4[:], b1r_sb[:])
                nc.vector.tensor_scalar_max(h4[:], h4[:], 0.0)
                sa = wpool.tile([128, C], F32, name="sa", tag="se_a")
                sb2 = wpool.tile([128, C], F32, name="sb2", tag="se_b")
                nc.vector.scalar_tensor_tensor(
                    sa[:], w2r_sb[:, 0, :], h4[:, 0:1], b2r_sb[:],
                    ALU.mult, ALU.add,
                )
                nc.vector.scalar_tensor_tensor(
                    sb2[:], w2r_sb[:, 1, :], h4[:, 1:2], sa[:], ALU.mult, ALU.add,
                )
                nc.vector.scalar_tensor_tensor(
                    sa[:], w2r_sb[:, 2, :], h4[:, 2:3], sb2[:], ALU.mult, ALU.add,
                )
                nc.vector.scalar_tensor_tensor(
                    sb2[:], w2r_sb[:, 3, :], h4[:, 3:4], sa[:], ALU.mult, ALU.add,
                )
                s_b = wpool.tile([128, C], F32, name="s_b", tag="se6")
                nc.scalar.activation(s_b[:], sb2[:], AF.Sigmoid)

                # ---- scale into fp32 staging + writeback (one DVE mul/group)
                for g in range(NG):
                    stg = spool.tile([128, GS, 2, W], F32, name="stg", tag="stg")
                    ssl = s_b[:, GS * g:GS * (g + 1)]
                    sbc = ssl.unsqueeze(-1).unsqueeze(-1).broadcast_to(
                        (128, GS, 2, W)
                    )
                    nc.vector.tensor_mul(stg[:], xt[b, g][:], sbc)
                    dst = out_d[b].rearrange("c (p k) w -> p c k w", p=128)[
                        :, GS * g:GS * (g + 1), :, :
                    ]
                    nc.sync.dma_start(dst, stg[:])

    nc.compile()
    return nc


_NC = None


def _get_nc():
    global _NC
    if _NC is None:
        _NC = _build_nc()
    return _NC


def _execute(inputs, trace=False):
    x = np.ascontiguousarray(np.asarray(inputs["x"], dtype=np.float32))
    consts = _build_consts(
        np.asarray(inputs["w1"]), np.asarray(inputs["b1"]),
        np.asarray(inputs["w2"]), np.asarray(inputs["b2"]),
    )
    in_maps = []
    for i in range(N_CORES):
        m = {"x": np.ascontiguousarray(x[BPC * i:BPC * (i + 1)])}
        m.update(consts)
        in_maps.append(m)
    nc = _get_nc()
    res = run_bass_kernel_spmd(nc, in_maps, core_ids=list(range(N_CORES)), trace=trace)
    out = np.concatenate([res.results[i]["out"] for i in range(N_CORES)], axis=0)
    return out, res


def kernel(x, w1, b1, w2, b2):
    out, _ = _execute({"x": x, "w1": w1, "b1": b1, "w2": w2, "b2": b2}, trace=False)
    return out
